# revision 18
# baseline (speedup 1.0000x reference)
"""Bronx GNN layer (semantic softmax attention + row-normalized gram mixing)
for 8 Trainium2 NeuronCores.

Strategy (per core, SPMD over row-shards of 1024 rows):
  prepass: LN(h) -> HN (bf16 resident), hnT windows -> qT (f32r, DRAM temp),
           x -> xb (bf16 resident) + xT (f32r, DRAM temp),
           own-shard kT/xT blocks (f32r resident).
  main loop per 256-row block:
    pass1 (per 128-col chunk): scoresT = qT_chunk.T @ kT_blk (f32r),
        gramT = xT_chunk.T @ xT_blk (f32r), exp via ACT (scale 1/sqrt(D)),
        square via ACT, stats S / G^2 via ones-matmul (broadcast over
        partitions), store expT/gT strips in bf16.
    mix: B_S = m/S (DVE recip), B_G = m/G via exp(-0.5 ln G^2) (ACT),
        A^T = expT*B_S + gT*B_G per chunk (DVE).
    pass2 (per chunk): accumulate (A_mix @ hn)^T and (A_mix @ x)^T via
        matmuls with natural-layout HN / xb chunks as lhsT (bf16).
    tail: @Wv (f32r) + ELU + h residual; PE-transpose x-path + x residual.

All inputs are full-size on every core; shard-specific tensors (h_blk,
x_blk) are sliced on the host so the device program is rank-agnostic.
"""

import numpy as np
from contextlib import ExitStack

N = 8192
D = 256
NCORES = 8
SHARD = N // NCORES          # 1024 rows per core
RB = 256                     # rows per row-block
NRB = SHARD // RB            # 4 row-blocks per core
NCH = N // 128               # 64 column chunks
LN_EPS = 1e-5
SCALE = float(D) ** -0.5


def _build(m00, m01, m10, m11, debug=False):
    import concourse.bass as bass
    import concourse.tile as tile
    from concourse import bacc, mybir
    from concourse.masks import make_identity

    fp32 = mybir.dt.float32
    f32r = mybir.dt.float32r
    bf16 = mybir.dt.bfloat16
    AF = mybir.ActivationFunctionType
    ALU = mybir.AluOpType

    fast_mix = abs(m00 - m01) < 1e-30 and abs(m10 - m11) < 1e-30

    nc = bacc.Bacc("TRN2", target_bir_lowering=False, debug=False)

    h_d = nc.dram_tensor("h", [N, D], fp32, kind="ExternalInput").ap()
    x_d = nc.dram_tensor("x", [N, D], fp32, kind="ExternalInput").ap()
    wk_d = nc.dram_tensor("Wk", [D, D], fp32, kind="ExternalInput").ap()
    wq_d = nc.dram_tensor("Wq", [D, D], fp32, kind="ExternalInput").ap()
    wv_d = nc.dram_tensor("Wv", [D, D], fp32, kind="ExternalInput").ap()
    gam_d = nc.dram_tensor("ln_gamma", [D], fp32, kind="ExternalInput").ap()
    bet_d = nc.dram_tensor("ln_beta", [D], fp32, kind="ExternalInput").ap()
    hb_d = nc.dram_tensor("h_blk", [SHARD, D], fp32, kind="ExternalInput").ap()
    xb_d = nc.dram_tensor("x_blk", [SHARD, D], fp32, kind="ExternalInput").ap()
    hout_d = nc.dram_tensor("h_new", [SHARD, D], fp32, kind="ExternalOutput").ap()
    xout_d = nc.dram_tensor("x_new", [SHARD, D], fp32, kind="ExternalOutput").ap()

    # DRAM temps: transposed q and x, chunk-major so main-loop loads are
    # contiguous 128KB blocks. [chunk, d_sub, d_in, n_in]
    qT_d = nc.dram_tensor("qT_tmp", [NCH, 2, 128, 128], f32r, kind="Internal").ap()
    xT_d = nc.dram_tensor("xT_tmp", [NCH, 2, 128, 128], f32r, kind="Internal").ap()

    dbg = {}
    if debug:
        for nm, shp in [("dbg_S", [128, RB]), ("dbg_G2", [128, RB]),
                        ("dbg_exp", [128, RB]), ("dbg_g", [128, RB]),
                        ("dbg_ah", [128, RB]), ("dbg_qt", [128, 2, 128]),
                        ("dbg_kt", [128, 2, RB]), ("dbg_hn", [128, D]),
                        ("dbg_ph", [128, 2, RB]), ("dbg_h3", [128, 2, RB]),
                        ("dbg_bsh", [128, RB]), ("dbg_bgh", [128, RB])]:
            dbg[nm] = nc.dram_tensor(nm, shp, fp32, kind="ExternalOutput").ap()

    with tile.TileContext(nc) as tc, ExitStack() as ctx:
        # ---------------- resident pools ----------------
        res = ctx.enter_context(tc.tile_pool(name="res", bufs=1))
        hn_res = res.tile([128, NCH, D], bf16)     # HN natural, bf16
        xn_res = res.tile([128, NCH, D], bf16)     # x natural, bf16
        ktb = res.tile([128, 2, SHARD], f32r)      # kT of own rows
        xtb = res.tile([128, 2, SHARD], f32r)      # xT of own rows
        wq_sb = res.tile([128, 2, D], f32r)
        wk_sb = res.tile([128, 2, D], f32r)
        wv_sb = res.tile([128, 2, D], f32r)
        ones_bf = res.tile([128, 128], bf16)
        ident = res.tile([128, 128], fp32)
        gam_b = res.tile([128, D], fp32)
        bet_b = res.tile([128, D], fp32)

        eps_c = res.tile([128, 1], fp32)
        tiny_c = res.tile([128, 1], fp32)
        nc.vector.memset(eps_c[:], LN_EPS)
        nc.vector.memset(tiny_c[:], 1e-30)
        nc.vector.memset(ones_bf[:], 1.0)
        make_identity(nc, ident)
        # broadcast gamma/beta across partitions via partition-step-0 DMA
        gam_bc = bass.AP(tensor=gam_d.tensor, offset=gam_d.offset,
                         ap=[[0, 128]] + list(gam_d.ap))
        bet_bc = bass.AP(tensor=bet_d.tensor, offset=bet_d.offset,
                         ap=[[0, 128]] + list(bet_d.ap))
        nc.sync.dma_start(gam_b[:], gam_bc)
        nc.sync.dma_start(bet_b[:], bet_bc)

        # weights -> f32r
        with tc.tile_pool(name="wtmp", bufs=1) as wtmp:
            for wd, wsb in ((wq_d, wq_sb), (wk_d, wk_sb), (wv_d, wv_sb)):
                wt = wtmp.tile([128, 2, D], fp32, tag="wt")
                nc.sync.dma_start(wt[:], wd.rearrange("(s p) d -> p s d", p=128))
                nc.vector.tensor_copy(wsb[:], wt[:])

        # ---------------- prepass ----------------
        def ln_chunk(pool, src_ap, gam_b, bet_b):
            """LayerNorm one [128, D] fp32 chunk; returns fp32 sbuf tile."""
            ht = pool.tile([128, D], fp32, tag="ln_h")
            nc.sync.dma_start(ht[:], src_ap)
            st = pool.tile([128, nc.vector.BN_STATS_DIM], fp32, tag="ln_st")
            nc.vector.bn_stats(out=st[:], in_=ht[:])
            mv = pool.tile([128, nc.vector.BN_AGGR_DIM], fp32, tag="ln_mv")
            nc.vector.bn_aggr(out=mv[:], in_=st[:])
            lt = pool.tile([128, 1], fp32, tag="ln_lt")
            nc.scalar.activation(lt[:], mv[:, 1:2], AF.Ln, bias=eps_c[:], scale=1.0)
            rstd = pool.tile([128, 1], fp32, tag="ln_rstd")
            nc.scalar.activation(rstd[:], lt[:], AF.Exp, bias=0.0, scale=-0.5)
            cen = pool.tile([128, D], fp32, tag="ln_cen")
            nc.vector.scalar_tensor_tensor(
                cen[:], ht[:], mv[:, 0:1], gam_b[:], ALU.subtract, ALU.mult)
            hnf = pool.tile([128, D], fp32, tag="ln_out")
            nc.vector.scalar_tensor_tensor(
                hnf[:], cen[:], rstd[:], bet_b[:], ALU.mult, ALU.add)
            return hnf

        with tc.tile_pool(name="pre", bufs=3) as pre, \
             tc.tile_pool(name="pre_win", bufs=2) as pre_win, \
             tc.tile_pool(name="pre_ps", bufs=2, space="PSUM") as pre_ps, \
             tc.tile_pool(name="pre_ps2", bufs=2, space="PSUM") as pre_ps2:

            def transpose_to(win_tile, i, fslice, src_fp32):
                """PE-transpose [128,128] fp32 -> f32r window slice."""
                pst = pre_ps.tile([128, 128], fp32, tag="tps")
                nc.tensor.transpose(pst[:], src_fp32, ident[:])
                nc.vector.tensor_copy(win_tile[:, i, fslice], pst[:])

            # full-N pass: HN resident + qT windows + xb resident + xT
            for w in range(NCH // 4):
                hnT_win = pre_win.tile([128, 2, 512], f32r, tag="hnT_win")
                xT_win = pre_win.tile([128, 2, 512], f32r, tag="xT_win")
                for jj in range(4):
                    j = w * 4 + jj
                    hnf = ln_chunk(pre, h_d[j * 128:(j + 1) * 128, :], gam_b, bet_b)
                    nc.vector.tensor_copy(hn_res[:, j, :], hnf[:])
                    fs = bass.ds(jj * 128, 128)
                    for i in range(2):
                        transpose_to(hnT_win, i, fs, hnf[:, i * 128:(i + 1) * 128])
                    xt = pre.tile([128, D], fp32, tag="x_in")
                    nc.sync.dma_start(xt[:], x_d[j * 128:(j + 1) * 128, :])
                    nc.vector.tensor_copy(xn_res[:, j, :], xt[:])
                    for i in range(2):
                        transpose_to(xT_win, i, fs, xt[:, i * 128:(i + 1) * 128])
                # qT = Wq^T @ hnT over this 512-col window
                psq = pre_ps2.tile([128, 2, 512], fp32, tag="psq")
                for p2 in range(2):
                    for i in range(2):
                        nc.tensor.matmul(
                            psq[:, p2, :], wq_sb[:, i, p2 * 128:(p2 + 1) * 128],
                            hnT_win[:, i, :], start=(i == 0), stop=(i == 1))
                qt_sb = pre_win.tile([128, 2, 512], f32r, tag="qt_sb")
                nc.vector.tensor_copy(qt_sb[:], psq[:])
                for jj in range(4):
                    j = w * 4 + jj
                    sl = bass.ds(jj * 128, 128)
                    nc.sync.dma_start(
                        qT_d[j].rearrange("s p n -> p s n"), qt_sb[:, :, sl])
                    nc.sync.dma_start(
                        xT_d[j].rearrange("s p n -> p s n"), xT_win[:, :, sl])

            # own-shard pass: kTb (needs LN of h_blk) and xTb
            for w in range(SHARD // 512):
                hnT_win = pre_win.tile([128, 2, 512], f32r, tag="hnT_win")
                for jj in range(4):
                    j = w * 4 + jj
                    hnf = ln_chunk(pre, hb_d[j * 128:(j + 1) * 128, :], gam_b, bet_b)
                    fs = bass.ds(jj * 128, 128)
                    for i in range(2):
                        transpose_to(hnT_win, i, fs, hnf[:, i * 128:(i + 1) * 128])
                    xt = pre.tile([128, D], fp32, tag="x_in")
                    nc.sync.dma_start(xt[:], xb_d[j * 128:(j + 1) * 128, :])
                    for i in range(2):
                        transpose_to(xtb, i, bass.ds(j * 128, 128),
                                     xt[:, i * 128:(i + 1) * 128])
                psq = pre_ps2.tile([128, 2, 512], fp32, tag="psq")
                for p2 in range(2):
                    for i in range(2):
                        nc.tensor.matmul(
                            psq[:, p2, :], wk_sb[:, i, p2 * 128:(p2 + 1) * 128],
                            hnT_win[:, i, :], start=(i == 0), stop=(i == 1))
                nc.vector.tensor_copy(ktb[:, :, w * 512:(w + 1) * 512], psq[:])

        # ---------------- main loop ----------------
        exp_pool = ctx.enter_context(tc.tile_pool(name="exp_strip", bufs=NCH))
        g_pool = ctx.enter_context(tc.tile_pool(name="g_strip", bufs=NCH))
        stream = ctx.enter_context(tc.tile_pool(name="stream", bufs=4))
        resid = ctx.enter_context(tc.tile_pool(name="resid", bufs=2))
        work = ctx.enter_context(tc.tile_pool(name="work", bufs=3))
        mixp = ctx.enter_context(tc.tile_pool(name="mixp", bufs=2))
        tailp = ctx.enter_context(tc.tile_pool(name="tailp", bufs=2))
        bpool = ctx.enter_context(tc.tile_pool(name="btiles", bufs=2))
        outp = ctx.enter_context(tc.tile_pool(name="outp", bufs=2))
        ps_sc = ctx.enter_context(tc.tile_pool(name="ps_sc", bufs=2, space="PSUM"))
        ps_st = ctx.enter_context(tc.tile_pool(name="ps_st", bufs=2, space="PSUM"))
        ps_ap = ctx.enter_context(tc.tile_pool(name="ps_ap", bufs=1, space="PSUM"))
        ps_tl = ctx.enter_context(tc.tile_pool(name="ps_tl", bufs=1, space="PSUM"))

        if debug:
            dbgp = ctx.enter_context(tc.tile_pool(name="dbgp", bufs=2))

            def dump(name, src_ap, shape):
                t = dbgp.tile(shape, fp32, tag="dump")
                nc.vector.tensor_copy(t[:], src_ap)
                nc.sync.dma_start(dbg[name], t[:])

            dump("dbg_hn", hn_res[:, 0, :], [128, D])
            dump("dbg_kt", ktb[:, :, 0:RB], [128, 2, RB])

        for rb in range(NRB):
            rsl = bass.ds(rb * RB, RB)        # row slice within shard

            # ---- pass 1: scores, exp, square, stats ----
            e_tiles, gt_tiles = [], []
            ps_stat = ps_st.tile([128, 2, RB], fp32, tag="stat")
            for j in range(NCH):
                qt = stream.tile([128, 2, 128], f32r, tag="qt")
                nc.sync.dma_start(qt[:], qT_d[j].rearrange("s p n -> p s n"))
                xt = stream.tile([128, 2, 128], f32r, tag="xt")
                nc.sync.dma_start(xt[:], xT_d[j].rearrange("s p n -> p s n"))

                psc = ps_sc.tile([128, 2, RB], fp32, tag="sc")
                for i in range(2):
                    nc.tensor.matmul(psc[:, 0, :], qt[:, i, :], ktb[:, i, rsl],
                                     start=(i == 0), stop=(i == 1))
                for i in range(2):
                    nc.tensor.matmul(psc[:, 1, :], xt[:, i, :], xtb[:, i, rsl],
                                     start=(i == 0), stop=(i == 1))

                et = exp_pool.tile([128, RB], bf16, tag="exp")
                nc.scalar.activation(et[:], psc[:, 0, :], AF.Exp,
                                     bias=0.0, scale=SCALE)
                gt = g_pool.tile([128, RB], bf16, tag="g")
                nc.vector.tensor_copy(gt[:], psc[:, 1, :])
                sq = work.tile([128, RB], bf16, tag="sq")
                nc.scalar.activation(sq[:], psc[:, 1, :], AF.Square)

                # ps_stat's two halves share one PSUM bank; start=True clears
                # has_written for the WHOLE bank, so only the first-touch
                # matmul of the bank may set it. The G2 half's first write
                # lands on cleared bits and overwrites, as intended.
                nc.tensor.matmul(ps_stat[:, 0, :], ones_bf[:], et[:],
                                 start=(j == 0), stop=(j == NCH - 1),
                                 skip_group_check=True)
                nc.tensor.matmul(ps_stat[:, 1, :], ones_bf[:], sq[:],
                                 start=False, stop=(j == NCH - 1),
                                 skip_group_check=True)
                if debug and rb == 0 and j == 0:
                    dump("dbg_exp", et[:], [128, RB])
                    dump("dbg_g", gt[:], [128, RB])
                    dump("dbg_qt", qt[:], [128, 2, 128])
                e_tiles.append(et)
                gt_tiles.append(gt)

            # ---- mix prep: B tiles ----
            b_sh = bpool.tile([128, RB], fp32, tag="b_sh")
            b_gh = bpool.tile([128, RB], fp32, tag="b_gh")
            rec = mixp.tile([128, RB], fp32, tag="rec")
            nc.vector.reciprocal(rec[:], ps_stat[:, 0, :])
            nc.vector.tensor_scalar_mul(b_sh[:], rec[:], float(m00))
            lng = mixp.tile([128, RB], fp32, tag="lng")
            nc.scalar.activation(lng[:], ps_stat[:, 1, :], AF.Ln,
                                 bias=tiny_c[:], scale=1.0)
            ginv = mixp.tile([128, RB], fp32, tag="ginv")
            nc.scalar.activation(ginv[:], lng[:], AF.Exp, bias=0.0, scale=-0.5)
            nc.vector.tensor_scalar_mul(b_gh[:], ginv[:], float(m10))
            if not fast_mix:
                b_sx = bpool.tile([128, RB], fp32, tag="b_sx")
                b_gx = bpool.tile([128, RB], fp32, tag="b_gx")
                nc.vector.tensor_scalar_mul(b_sx[:], rec[:], float(m01))
                nc.vector.tensor_scalar_mul(b_gx[:], ginv[:], float(m11))
            if debug and rb == 0:
                dump("dbg_S", ps_stat[:, 0, :], [128, RB])
                dump("dbg_G2", ps_stat[:, 1, :], [128, RB])
                dump("dbg_bsh", b_sh[:], [128, RB])
                dump("dbg_bgh", b_gh[:], [128, RB])

            # ---- pass 2: mix + apply ----
            ph = ps_ap.tile([128, 2, RB], fp32, tag="ph")
            px = ps_ap.tile([128, 2, RB], fp32, tag="px")
            for j in range(NCH):
                et, gt = e_tiles[j], gt_tiles[j]
                t1 = work.tile([128, RB], bf16, tag="t1")
                nc.vector.tensor_tensor(t1[:], et[:], b_sh[:], ALU.mult)
                t2 = work.tile([128, RB], bf16, tag="t2")
                nc.vector.tensor_tensor(t2[:], gt[:], b_gh[:], ALU.mult)
                ah = et  # reuse exp slot for mixed A_h
                if not fast_mix:
                    t1x = work.tile([128, RB], bf16, tag="t1x")
                    nc.vector.tensor_tensor(t1x[:], et[:], b_sx[:], ALU.mult)
                    t2x = work.tile([128, RB], bf16, tag="t2x")
                    nc.vector.tensor_tensor(t2x[:], gt[:], b_gx[:], ALU.mult)
                    ax = gt
                    nc.vector.tensor_tensor(ah[:], t1[:], t2[:], ALU.add)
                    nc.vector.tensor_tensor(ax[:], t1x[:], t2x[:], ALU.add)
                else:
                    ax = ah
                    nc.vector.tensor_tensor(ah[:], t1[:], t2[:], ALU.add)

                if debug and rb == 0 and j == 0:
                    dump("dbg_ah", ah[:], [128, RB])
                # ph (and px) halves share a bank: first-touch start only.
                for i in range(2):
                    nc.tensor.matmul(
                        ph[:, i, :], hn_res[:, j, i * 128:(i + 1) * 128], ah[:],
                        start=(j == 0 and i == 0), stop=(j == NCH - 1),
                        skip_group_check=True)
                for i in range(2):
                    nc.tensor.matmul(
                        px[:, i, :], xn_res[:, j, i * 128:(i + 1) * 128], ax[:],
                        start=(j == 0 and i == 0), stop=(j == NCH - 1),
                        skip_group_check=True)

            # ---- tail: h path (@Wv, ELU, residual) ----
            if debug and rb == 0:
                dump("dbg_ph", ph[:], [128, 2, RB])
            o2t = tailp.tile([128, 2, RB], f32r, tag="o2t")
            nc.vector.tensor_copy(o2t[:], ph[:])
            ps_h3 = ps_tl.tile([128, 2, RB], fp32, tag="h3")
            for rc in range(2):
                for dc in range(2):
                    nc.tensor.matmul(
                        ps_h3[:, rc, :], o2t[:, dc, rc * 128:(rc + 1) * 128],
                        wv_sb[:, dc, :], start=(dc == 0), stop=(dc == 1))
            if debug and rb == 0:
                dump("dbg_h3", ps_h3[:], [128, 2, RB])
            for rc in range(2):
                rows = bass.ds(rb * RB + rc * 128, 128)
                h0 = resid.tile([128, D], fp32, tag="h0")
                nc.sync.dma_start(h0[:], hb_d[rows, :])
                tmin = tailp.tile([128, D], fp32, tag="tmin")
                nc.vector.tensor_scalar_min(tmin[:], ps_h3[:, rc, :], 0.0)
                ee = tailp.tile([128, D], fp32, tag="ee")
                nc.scalar.activation(ee[:], tmin[:], AF.Exp)
                uu = tailp.tile([128, D], fp32, tag="uu")
                nc.vector.tensor_scalar_max(uu[:], ps_h3[:, rc, :], 0.0)
                s1 = tailp.tile([128, D], fp32, tag="s1")
                nc.vector.scalar_tensor_tensor(
                    s1[:], ee[:], -1.0, uu[:], ALU.add, ALU.add)
                ho = outp.tile([128, D], fp32, tag="hout")
                nc.vector.tensor_tensor(ho[:], s1[:], h0[:], ALU.add)
                nc.sync.dma_start(hout_d[rows, :], ho[:])

            # ---- tail: x path (transpose back, residual) ----
            oxt = tailp.tile([128, 2, RB], fp32, tag="oxt")
            nc.vector.tensor_copy(oxt[:], px[:])
            for rc in range(2):
                rows = bass.ds(rb * RB + rc * 128, 128)
                x0 = resid.tile([128, D], fp32, tag="x0")
                nc.sync.dma_start(x0[:], xb_d[rows, :])
                xo = outp.tile([128, D], fp32, tag="xout")
                for dc in range(2):
                    ps_tr = ps_tl.tile([128, 128], fp32, tag="tr")
                    nc.tensor.transpose(
                        ps_tr[:], oxt[:, dc, rc * 128:(rc + 1) * 128], ident[:])
                    nc.vector.tensor_tensor(
                        xo[:, dc * 128:(dc + 1) * 128], ps_tr[:],
                        x0[:, dc * 128:(dc + 1) * 128], ALU.add)
                nc.sync.dma_start(xout_d[rows, :], xo[:])

    nc.compile()
    return nc


_CACHE = {}


def kernel(h, x, Wk, Wq, Wv, ln_gamma, ln_beta, mixing):
    from concourse import bass_utils

    h = np.ascontiguousarray(h, dtype=np.float32)
    x = np.ascontiguousarray(x, dtype=np.float32)
    Wk = np.ascontiguousarray(Wk, dtype=np.float32)
    Wq = np.ascontiguousarray(Wq, dtype=np.float32)
    Wv = np.ascontiguousarray(Wv, dtype=np.float32)
    ln_gamma = np.ascontiguousarray(ln_gamma, dtype=np.float32)
    ln_beta = np.ascontiguousarray(ln_beta, dtype=np.float32)
    mixing = np.asarray(mixing, dtype=np.float64)

    # softmax over dim 0 of the 2x2 mixing (4 scalars -> compile-time consts)
    mx = mixing - mixing.max(axis=0, keepdims=True)
    em = np.exp(mx)
    m = em / em.sum(axis=0, keepdims=True)
    m00, m01, m10, m11 = float(m[0, 0]), float(m[0, 1]), float(m[1, 0]), float(m[1, 1])

    key = (m00, m01, m10, m11)
    if key not in _CACHE:
        _CACHE[key] = _build(m00, m01, m10, m11)
    nc = _CACHE[key]

    in_maps = []
    for c in range(NCORES):
        sl = slice(c * SHARD, (c + 1) * SHARD)
        in_maps.append({
            "h": h, "x": x, "Wk": Wk, "Wq": Wq, "Wv": Wv,
            "ln_gamma": ln_gamma, "ln_beta": ln_beta,
            "h_blk": np.ascontiguousarray(h[sl]),
            "x_blk": np.ascontiguousarray(x[sl]),
        })

    res = bass_utils.run_bass_kernel_spmd(
        nc, in_maps, core_ids=list(range(NCORES)))

    h_new = np.concatenate([res.results[c]["h_new"] for c in range(NCORES)], axis=0)
    x_new = np.concatenate([res.results[c]["x_new"] for c in range(NCORES)], axis=0)
    return (h_new, x_new)


# revision 24
# speedup vs baseline: 1.0137x; 1.0137x over previous
"""Bronx GNN layer (semantic softmax attention + row-normalized gram mixing)
for 8 Trainium2 NeuronCores.

Strategy (per core, SPMD over row-shards of 1024 rows):
  prepass: LN(h) -> HN (bf16 resident), hnT windows -> qT (f32r, DRAM temp),
           x -> xb (bf16 resident) + xT (f32r, DRAM temp),
           own-shard kT/xT blocks (f32r resident).
  main loop per 256-row block:
    pass1 (per 128-col chunk): scoresT = qT_chunk.T @ kT_blk (f32r),
        gramT = xT_chunk.T @ xT_blk (f32r), exp via ACT (scale 1/sqrt(D)),
        square via ACT, stats S / G^2 via ones-matmul (broadcast over
        partitions), store expT/gT strips in bf16.
    mix: B_S = m/S (DVE recip), B_G = m/G via exp(-0.5 ln G^2) (ACT),
        A^T = expT*B_S + gT*B_G per chunk (DVE).
    pass2 (per chunk): accumulate (A_mix @ hn)^T and (A_mix @ x)^T via
        matmuls with natural-layout HN / xb chunks as lhsT (bf16).
    tail: @Wv (f32r) + ELU + h residual; PE-transpose x-path + x residual.

All inputs are full-size on every core; shard-specific tensors (h_blk,
x_blk) are sliced on the host so the device program is rank-agnostic.
"""

import numpy as np
from contextlib import ExitStack

N = 8192
D = 256
NCORES = 8
SHARD = N // NCORES          # 1024 rows per core
RB = 256                     # rows per row-block
NRB = SHARD // RB            # 4 row-blocks per core
NCH = N // 128               # 64 column chunks
LN_EPS = 1e-5
SCALE = float(D) ** -0.5


def _build(m00, m01, m10, m11, debug=False):
    import concourse.bass as bass
    import concourse.tile as tile
    from concourse import bacc, mybir
    from concourse.masks import make_identity

    fp32 = mybir.dt.float32
    f32r = mybir.dt.float32r
    bf16 = mybir.dt.bfloat16
    AF = mybir.ActivationFunctionType
    ALU = mybir.AluOpType

    fast_mix = abs(m00 - m01) < 1e-30 and abs(m10 - m11) < 1e-30

    nc = bacc.Bacc("TRN2", target_bir_lowering=False, debug=False)

    h_d = nc.dram_tensor("h", [N, D], fp32, kind="ExternalInput").ap()
    x_d = nc.dram_tensor("x", [N, D], fp32, kind="ExternalInput").ap()
    wk_d = nc.dram_tensor("Wk", [D, D], fp32, kind="ExternalInput").ap()
    wq_d = nc.dram_tensor("Wq", [D, D], fp32, kind="ExternalInput").ap()
    wv_d = nc.dram_tensor("Wv", [D, D], fp32, kind="ExternalInput").ap()
    gam_d = nc.dram_tensor("ln_gamma", [D], fp32, kind="ExternalInput").ap()
    bet_d = nc.dram_tensor("ln_beta", [D], fp32, kind="ExternalInput").ap()
    hb_d = nc.dram_tensor("h_blk", [SHARD, D], fp32, kind="ExternalInput").ap()
    xb_d = nc.dram_tensor("x_blk", [SHARD, D], fp32, kind="ExternalInput").ap()
    hout_d = nc.dram_tensor("h_new", [SHARD, D], fp32, kind="ExternalOutput").ap()
    xout_d = nc.dram_tensor("x_new", [SHARD, D], fp32, kind="ExternalOutput").ap()

    # DRAM temps: transposed q and x, chunk-major so main-loop loads are
    # contiguous 128KB blocks. [chunk, d_sub, d_in, n_in]
    qT_d = nc.dram_tensor("qT_tmp", [NCH, 2, 128, 128], f32r, kind="Internal").ap()
    xT_d = nc.dram_tensor("xT_tmp", [NCH, 2, 128, 128], f32r, kind="Internal").ap()

    dbg = {}
    if debug:
        for nm, shp in [("dbg_S", [128, RB]), ("dbg_G2", [128, RB]),
                        ("dbg_exp", [128, RB]), ("dbg_g", [128, RB]),
                        ("dbg_ah", [128, RB]), ("dbg_qt", [128, 2, 128]),
                        ("dbg_kt", [128, 2, RB]), ("dbg_hn", [128, D]),
                        ("dbg_ph", [128, 2, RB]), ("dbg_h3", [128, 2, RB]),
                        ("dbg_bsh", [128, RB]), ("dbg_bgh", [128, RB])]:
            dbg[nm] = nc.dram_tensor(nm, shp, fp32, kind="ExternalOutput").ap()

    with tile.TileContext(nc) as tc, ExitStack() as ctx:
        # ---------------- resident pools ----------------
        res = ctx.enter_context(tc.tile_pool(name="res", bufs=1))
        hn_res = res.tile([128, NCH, D], bf16)     # HN natural, bf16
        xn_res = res.tile([128, NCH, D], bf16)     # x natural, bf16
        ktb = res.tile([128, 2, SHARD], f32r)      # kT of own rows
        xtb = res.tile([128, 2, SHARD], f32r)      # xT of own rows
        wq_sb = res.tile([128, 2, D], f32r)
        wk_sb = res.tile([128, 2, D], f32r)
        wv_sb = res.tile([128, 2, D], f32r)
        ones_bf = res.tile([128, 128], bf16)
        ident = res.tile([128, 128], fp32)
        gam_b = res.tile([128, D], fp32)
        bet_b = res.tile([128, D], fp32)

        eps_c = res.tile([128, 1], fp32)
        tiny_c = res.tile([128, 1], fp32)
        nc.vector.memset(eps_c[:], LN_EPS)
        nc.vector.memset(tiny_c[:], 1e-30)
        nc.vector.memset(ones_bf[:], 1.0)
        make_identity(nc, ident)
        # broadcast gamma/beta across partitions via partition-step-0 DMA
        gam_bc = bass.AP(tensor=gam_d.tensor, offset=gam_d.offset,
                         ap=[[0, 128]] + list(gam_d.ap))
        bet_bc = bass.AP(tensor=bet_d.tensor, offset=bet_d.offset,
                         ap=[[0, 128]] + list(bet_d.ap))
        nc.sync.dma_start(gam_b[:], gam_bc)
        nc.sync.dma_start(bet_b[:], bet_bc)

        # weights -> f32r
        with tc.tile_pool(name="wtmp", bufs=1) as wtmp:
            for wd, wsb in ((wq_d, wq_sb), (wk_d, wk_sb), (wv_d, wv_sb)):
                wt = wtmp.tile([128, 2, D], fp32, tag="wt")
                nc.sync.dma_start(wt[:], wd.rearrange("(s p) d -> p s d", p=128))
                nc.vector.tensor_copy(wsb[:], wt[:])

        # ---------------- prepass ----------------
        # LN stats for all 64 full-N chunks + 8 own-shard chunks in one
        # sweep, then a SINGLE batched Ln + Exp for every rstd — doing
        # Ln/Exp per chunk thrashes the ACT table sets (~2.7us per switch).
        NLN = NCH + SHARD // 128
        mv_res = res.tile([128, NLN, 2], fp32)
        rstd_res = res.tile([128, NLN], fp32)

        with tc.tile_pool(name="lnstat", bufs=4) as lnp:
            for j in range(NLN):
                src = (h_d[j * 128:(j + 1) * 128, :] if j < NCH
                       else hb_d[(j - NCH) * 128:(j - NCH + 1) * 128, :])
                ht = lnp.tile([128, D], fp32, tag="ln_h")
                nc.sync.dma_start(ht[:], src)
                st = lnp.tile([128, nc.vector.BN_STATS_DIM], fp32, tag="ln_st")
                nc.vector.bn_stats(out=st[:], in_=ht[:])
                nc.vector.bn_aggr(out=mv_res[:, j, :], in_=st[:])
            lnt = lnp.tile([128, NLN], fp32, tag="ln_t")
            nc.scalar.activation(lnt[:], mv_res[:, :, 1], AF.Ln,
                                 bias=eps_c[:], scale=1.0)
            nc.scalar.activation(rstd_res[:], lnt[:], AF.Exp,
                                 bias=0.0, scale=-0.5)

        def ln_chunk(pool, src_ap, mvj, rstdj):
            """Re-load chunk and normalize with precomputed stats (no ACT)."""
            ht = pool.tile([128, D], fp32, tag="ln_h")
            nc.sync.dma_start(ht[:], src_ap)
            cen = pool.tile([128, D], fp32, tag="ln_cen")
            nc.vector.scalar_tensor_tensor(
                cen[:], ht[:], mvj, gam_b[:], ALU.subtract, ALU.mult)
            hnf = pool.tile([128, D], fp32, tag="ln_out")
            nc.vector.scalar_tensor_tensor(
                hnf[:], cen[:], rstdj, bet_b[:], ALU.mult, ALU.add)
            return hnf

        with tc.tile_pool(name="pre", bufs=3) as pre, \
             tc.tile_pool(name="pre_win", bufs=2) as pre_win, \
             tc.tile_pool(name="pre_ps", bufs=2, space="PSUM") as pre_ps, \
             tc.tile_pool(name="pre_ps2", bufs=2, space="PSUM") as pre_ps2:

            def transpose_to(win_tile, i, fslice, src_fp32):
                """PE-transpose [128,128] fp32 -> f32r window slice."""
                pst = pre_ps.tile([128, 128], fp32, tag="tps")
                nc.tensor.transpose(pst[:], src_fp32, ident[:])
                nc.vector.tensor_copy(win_tile[:, i, fslice], pst[:])

            # full-N pass: HN resident + qT windows + xb resident + xT
            for w in range(NCH // 4):
                hnT_win = pre_win.tile([128, 2, 512], f32r, tag="hnT_win")
                xT_win = pre_win.tile([128, 2, 512], f32r, tag="xT_win")
                for jj in range(4):
                    j = w * 4 + jj
                    hnf = ln_chunk(pre, h_d[j * 128:(j + 1) * 128, :],
                                   mv_res[:, j, 0:1], rstd_res[:, j:j + 1])
                    nc.vector.tensor_copy(hn_res[:, j, :], hnf[:])
                    fs = bass.ds(jj * 128, 128)
                    for i in range(2):
                        transpose_to(hnT_win, i, fs, hnf[:, i * 128:(i + 1) * 128])
                    xt = pre.tile([128, D], fp32, tag="x_in")
                    nc.sync.dma_start(xt[:], x_d[j * 128:(j + 1) * 128, :])
                    nc.vector.tensor_copy(xn_res[:, j, :], xt[:])
                    for i in range(2):
                        transpose_to(xT_win, i, fs, xt[:, i * 128:(i + 1) * 128])
                # qT = Wq^T @ hnT over this 512-col window
                psq = pre_ps2.tile([128, 2, 512], fp32, tag="psq")
                for p2 in range(2):
                    for i in range(2):
                        nc.tensor.matmul(
                            psq[:, p2, :], wq_sb[:, i, p2 * 128:(p2 + 1) * 128],
                            hnT_win[:, i, :], start=(i == 0), stop=(i == 1))
                qt_sb = pre_win.tile([128, 2, 512], f32r, tag="qt_sb")
                nc.vector.tensor_copy(qt_sb[:], psq[:])
                for jj in range(4):
                    j = w * 4 + jj
                    sl = bass.ds(jj * 128, 128)
                    nc.sync.dma_start(
                        qT_d[j].rearrange("s p n -> p s n"), qt_sb[:, :, sl])
                    nc.sync.dma_start(
                        xT_d[j].rearrange("s p n -> p s n"), xT_win[:, :, sl])

            # own-shard pass: kTb (needs LN of h_blk) and xTb
            for w in range(SHARD // 512):
                hnT_win = pre_win.tile([128, 2, 512], f32r, tag="hnT_win")
                for jj in range(4):
                    j = w * 4 + jj
                    hnf = ln_chunk(pre, hb_d[j * 128:(j + 1) * 128, :],
                                   mv_res[:, NCH + j, 0:1],
                                   rstd_res[:, NCH + j:NCH + j + 1])
                    fs = bass.ds(jj * 128, 128)
                    for i in range(2):
                        transpose_to(hnT_win, i, fs, hnf[:, i * 128:(i + 1) * 128])
                    xt = pre.tile([128, D], fp32, tag="x_in")
                    nc.sync.dma_start(xt[:], xb_d[j * 128:(j + 1) * 128, :])
                    for i in range(2):
                        transpose_to(xtb, i, bass.ds(j * 128, 128),
                                     xt[:, i * 128:(i + 1) * 128])
                psq = pre_ps2.tile([128, 2, 512], fp32, tag="psq")
                for p2 in range(2):
                    for i in range(2):
                        nc.tensor.matmul(
                            psq[:, p2, :], wk_sb[:, i, p2 * 128:(p2 + 1) * 128],
                            hnT_win[:, i, :], start=(i == 0), stop=(i == 1))
                nc.vector.tensor_copy(ktb[:, :, w * 512:(w + 1) * 512], psq[:])

        # ---------------- main loop ----------------
        exp_pool = ctx.enter_context(tc.tile_pool(name="exp_strip", bufs=NCH))
        g_pool = ctx.enter_context(tc.tile_pool(name="g_strip", bufs=NCH))
        stream = ctx.enter_context(tc.tile_pool(name="stream", bufs=6))
        resid = ctx.enter_context(tc.tile_pool(name="resid", bufs=2))
        work = ctx.enter_context(tc.tile_pool(name="work", bufs=3))
        mixp = ctx.enter_context(tc.tile_pool(name="mixp", bufs=2))
        tailp = ctx.enter_context(tc.tile_pool(name="tailp", bufs=2))
        bpool = ctx.enter_context(tc.tile_pool(name="btiles", bufs=2))
        outp = ctx.enter_context(tc.tile_pool(name="outp", bufs=2))
        ps_sc = ctx.enter_context(tc.tile_pool(name="ps_sc", bufs=3, space="PSUM"))
        ps_st = ctx.enter_context(tc.tile_pool(name="ps_st", bufs=1, space="PSUM"))
        ps_ap = ctx.enter_context(tc.tile_pool(name="ps_ap", bufs=1, space="PSUM"))
        ps_tl = ctx.enter_context(tc.tile_pool(name="ps_tl", bufs=1, space="PSUM"))

        if debug:
            dbgp = ctx.enter_context(tc.tile_pool(name="dbgp", bufs=2))

            def dump(name, src_ap, shape):
                t = dbgp.tile(shape, fp32, tag="dump")
                nc.vector.tensor_copy(t[:], src_ap)
                nc.sync.dma_start(dbg[name], t[:])

            dump("dbg_hn", hn_res[:, 0, :], [128, D])
            dump("dbg_kt", ktb[:, :, 0:RB], [128, 2, RB])

        for rb in range(NRB):
            rsl = bass.ds(rb * RB, RB)        # row slice within shard

            # ---- pass 1: scores, exp, square, stats ----
            e_tiles, gt_tiles = [], []
            ps_stat = ps_st.tile([128, 2, RB], fp32, tag="stat")
            for j in range(NCH):
                qt = stream.tile([128, 2, 128], f32r, tag="qt")
                nc.sync.dma_start(qt[:], qT_d[j].rearrange("s p n -> p s n"))
                xt = stream.tile([128, 2, 128], f32r, tag="xt")
                nc.sync.dma_start(xt[:], xT_d[j].rearrange("s p n -> p s n"))

                psc = ps_sc.tile([128, 2, RB], fp32, tag="sc")
                for i in range(2):
                    nc.tensor.matmul(psc[:, 0, :], qt[:, i, :], ktb[:, i, rsl],
                                     start=(i == 0), stop=(i == 1))
                for i in range(2):
                    nc.tensor.matmul(psc[:, 1, :], xt[:, i, :], xtb[:, i, rsl],
                                     start=(i == 0), stop=(i == 1))

                et = exp_pool.tile([128, RB], bf16, tag="exp")
                nc.scalar.activation(et[:], psc[:, 0, :], AF.Exp,
                                     bias=0.0, scale=SCALE)
                gt = g_pool.tile([128, RB], bf16, tag="g")
                nc.vector.tensor_copy(gt[:], psc[:, 1, :])
                sq = work.tile([128, RB], bf16, tag="sq")
                nc.scalar.activation(sq[:], psc[:, 1, :], AF.Square)

                # ps_stat's two halves share one PSUM bank; start=True clears
                # has_written for the WHOLE bank, so only the first-touch
                # matmul of the bank may set it. The G2 half's first write
                # lands on cleared bits and overwrites, as intended.
                nc.tensor.matmul(ps_stat[:, 0, :], ones_bf[:], et[:],
                                 start=(j == 0), stop=(j == NCH - 1),
                                 skip_group_check=True)
                nc.tensor.matmul(ps_stat[:, 1, :], ones_bf[:], sq[:],
                                 start=False, stop=(j == NCH - 1),
                                 skip_group_check=True)
                if debug and rb == 0 and j == 0:
                    dump("dbg_exp", et[:], [128, RB])
                    dump("dbg_g", gt[:], [128, RB])
                    dump("dbg_qt", qt[:], [128, 2, 128])
                e_tiles.append(et)
                gt_tiles.append(gt)

            # ---- mix prep: B tiles ----
            b_sh = bpool.tile([128, RB], fp32, tag="b_sh")
            b_gh = bpool.tile([128, RB], fp32, tag="b_gh")
            rec = mixp.tile([128, RB], fp32, tag="rec")
            nc.vector.reciprocal(rec[:], ps_stat[:, 0, :])
            nc.vector.tensor_scalar_mul(b_sh[:], rec[:], float(m00))
            lng = mixp.tile([128, RB], fp32, tag="lng")
            nc.scalar.activation(lng[:], ps_stat[:, 1, :], AF.Ln,
                                 bias=tiny_c[:], scale=1.0)
            ginv = mixp.tile([128, RB], fp32, tag="ginv")
            nc.scalar.activation(ginv[:], lng[:], AF.Exp, bias=0.0, scale=-0.5)
            nc.vector.tensor_scalar_mul(b_gh[:], ginv[:], float(m10))
            if not fast_mix:
                b_sx = bpool.tile([128, RB], fp32, tag="b_sx")
                b_gx = bpool.tile([128, RB], fp32, tag="b_gx")
                nc.vector.tensor_scalar_mul(b_sx[:], rec[:], float(m01))
                nc.vector.tensor_scalar_mul(b_gx[:], ginv[:], float(m11))
            if debug and rb == 0:
                dump("dbg_S", ps_stat[:, 0, :], [128, RB])
                dump("dbg_G2", ps_stat[:, 1, :], [128, RB])
                dump("dbg_bsh", b_sh[:], [128, RB])
                dump("dbg_bgh", b_gh[:], [128, RB])

            # ---- pass 2: mix + apply ----
            ph = ps_ap.tile([128, 2, RB], fp32, tag="ph")
            px = ps_ap.tile([128, 2, RB], fp32, tag="px")
            for j in range(NCH):
                et, gt = e_tiles[j], gt_tiles[j]
                t1 = work.tile([128, RB], bf16, tag="t1")
                nc.vector.tensor_tensor(t1[:], et[:], b_sh[:], ALU.mult)
                t2 = work.tile([128, RB], bf16, tag="t2")
                nc.gpsimd.tensor_tensor(t2[:], gt[:], b_gh[:], ALU.mult)
                ah = et  # reuse exp slot for mixed A_h
                if not fast_mix:
                    t1x = work.tile([128, RB], bf16, tag="t1x")
                    nc.vector.tensor_tensor(t1x[:], et[:], b_sx[:], ALU.mult)
                    t2x = work.tile([128, RB], bf16, tag="t2x")
                    nc.vector.tensor_tensor(t2x[:], gt[:], b_gx[:], ALU.mult)
                    ax = gt
                    nc.vector.tensor_tensor(ah[:], t1[:], t2[:], ALU.add)
                    nc.vector.tensor_tensor(ax[:], t1x[:], t2x[:], ALU.add)
                else:
                    ax = ah
                    nc.vector.tensor_tensor(ah[:], t1[:], t2[:], ALU.add)

                if debug and rb == 0 and j == 0:
                    dump("dbg_ah", ah[:], [128, RB])
                # ph (and px) halves share a bank: first-touch start only.
                for i in range(2):
                    nc.tensor.matmul(
                        ph[:, i, :], hn_res[:, j, i * 128:(i + 1) * 128], ah[:],
                        start=(j == 0 and i == 0), stop=(j == NCH - 1),
                        skip_group_check=True)
                for i in range(2):
                    nc.tensor.matmul(
                        px[:, i, :], xn_res[:, j, i * 128:(i + 1) * 128], ax[:],
                        start=(j == 0 and i == 0), stop=(j == NCH - 1),
                        skip_group_check=True)

            # ---- tail: h path (@Wv, ELU, residual) ----
            if debug and rb == 0:
                dump("dbg_ph", ph[:], [128, 2, RB])
            o2t = tailp.tile([128, 2, RB], f32r, tag="o2t")
            nc.vector.tensor_copy(o2t[:], ph[:])
            ps_h3 = ps_tl.tile([128, 2, RB], fp32, tag="h3")
            for rc in range(2):
                for dc in range(2):
                    nc.tensor.matmul(
                        ps_h3[:, rc, :], o2t[:, dc, rc * 128:(rc + 1) * 128],
                        wv_sb[:, dc, :], start=(dc == 0), stop=(dc == 1))
            if debug and rb == 0:
                dump("dbg_h3", ps_h3[:], [128, 2, RB])
            for rc in range(2):
                rows = bass.ds(rb * RB + rc * 128, 128)
                h0 = resid.tile([128, D], fp32, tag="h0")
                nc.sync.dma_start(h0[:], hb_d[rows, :])
                tmin = tailp.tile([128, D], fp32, tag="tmin")
                nc.vector.tensor_scalar_min(tmin[:], ps_h3[:, rc, :], 0.0)
                ee = tailp.tile([128, D], fp32, tag="ee")
                nc.scalar.activation(ee[:], tmin[:], AF.Exp)
                uu = tailp.tile([128, D], fp32, tag="uu")
                nc.vector.tensor_scalar_max(uu[:], ps_h3[:, rc, :], 0.0)
                s1 = tailp.tile([128, D], fp32, tag="s1")
                nc.vector.scalar_tensor_tensor(
                    s1[:], ee[:], -1.0, uu[:], ALU.add, ALU.add)
                ho = outp.tile([128, D], fp32, tag="hout")
                nc.vector.tensor_tensor(ho[:], s1[:], h0[:], ALU.add)
                nc.sync.dma_start(hout_d[rows, :], ho[:])

            # ---- tail: x path (transpose back, residual) ----
            oxt = tailp.tile([128, 2, RB], fp32, tag="oxt")
            nc.vector.tensor_copy(oxt[:], px[:])
            for rc in range(2):
                rows = bass.ds(rb * RB + rc * 128, 128)
                x0 = resid.tile([128, D], fp32, tag="x0")
                nc.sync.dma_start(x0[:], xb_d[rows, :])
                xo = outp.tile([128, D], fp32, tag="xout")
                for dc in range(2):
                    ps_tr = ps_tl.tile([128, 128], fp32, tag="tr")
                    nc.tensor.transpose(
                        ps_tr[:], oxt[:, dc, rc * 128:(rc + 1) * 128], ident[:])
                    nc.vector.tensor_tensor(
                        xo[:, dc * 128:(dc + 1) * 128], ps_tr[:],
                        x0[:, dc * 128:(dc + 1) * 128], ALU.add)
                nc.sync.dma_start(xout_d[rows, :], xo[:])

    nc.compile()
    return nc


_CACHE = {}


def kernel(h, x, Wk, Wq, Wv, ln_gamma, ln_beta, mixing):
    from concourse import bass_utils

    h = np.ascontiguousarray(h, dtype=np.float32)
    x = np.ascontiguousarray(x, dtype=np.float32)
    Wk = np.ascontiguousarray(Wk, dtype=np.float32)
    Wq = np.ascontiguousarray(Wq, dtype=np.float32)
    Wv = np.ascontiguousarray(Wv, dtype=np.float32)
    ln_gamma = np.ascontiguousarray(ln_gamma, dtype=np.float32)
    ln_beta = np.ascontiguousarray(ln_beta, dtype=np.float32)
    mixing = np.asarray(mixing, dtype=np.float64)

    # softmax over dim 0 of the 2x2 mixing (4 scalars -> compile-time consts)
    mx = mixing - mixing.max(axis=0, keepdims=True)
    em = np.exp(mx)
    m = em / em.sum(axis=0, keepdims=True)
    m00, m01, m10, m11 = float(m[0, 0]), float(m[0, 1]), float(m[1, 0]), float(m[1, 1])

    key = (m00, m01, m10, m11)
    if key not in _CACHE:
        _CACHE[key] = _build(m00, m01, m10, m11)
    nc = _CACHE[key]

    in_maps = []
    for c in range(NCORES):
        sl = slice(c * SHARD, (c + 1) * SHARD)
        in_maps.append({
            "h": h, "x": x, "Wk": Wk, "Wq": Wq, "Wv": Wv,
            "ln_gamma": ln_gamma, "ln_beta": ln_beta,
            "h_blk": np.ascontiguousarray(h[sl]),
            "x_blk": np.ascontiguousarray(x[sl]),
        })

    res = bass_utils.run_bass_kernel_spmd(
        nc, in_maps, core_ids=list(range(NCORES)))

    h_new = np.concatenate([res.results[c]["h_new"] for c in range(NCORES)], axis=0)
    x_new = np.concatenate([res.results[c]["x_new"] for c in range(NCORES)], axis=0)
    return (h_new, x_new)


# revision 28
# speedup vs baseline: 1.2673x; 1.2502x over previous
"""Bronx GNN layer (semantic softmax attention + row-normalized gram mixing)
for 8 Trainium2 NeuronCores.

Strategy (per core, SPMD over row-shards of 1024 rows):
  prepass: batched LN stats (one Ln/Exp for all chunks -> no ACT table
           thrash), LN(h) -> HN (bf16 resident), hnT windows -> qT (f32r,
           DRAM temp), x -> xb (bf16 resident) + xT (f32r, DRAM temp),
           own-shard kT/xT blocks (f32r resident).
  main loop per 256-row block, chunks processed in PAIRS so ACT/DVE ops
  run at [128, 512] width (amortizes per-op fixed overhead):
    pass1: scoresT = qT_chunk.T @ kT_blk (f32r), gramT = xT_chunk.T @
        xT_blk (f32r), exp via ACT (scale 1/sqrt(D)), square via ACT,
        stats S / G^2 via ones-matmul (broadcast over partitions),
        expT/gT strips in bf16.
    mix: B_S = m/S (DVE recip), B_G = m/G via exp(-0.5 ln G^2) (ACT),
        A^T = expT*B_S + gT*B_G (DVE + GpSimd).
    pass2: accumulate (A_mix @ hn)^T and (A_mix @ x)^T via matmuls with
        natural-layout HN / xb chunks (bf16) as lhsT.
    tail: @Wv (f32r) + ELU + h residual; PE-transpose x path + residual.

PSUM accumulator halves that share a bank use first-touch-only start=True
(start clears has_written for the WHOLE bank).

All inputs are full-size on every core; shard-specific tensors (h_blk,
x_blk) are sliced on the host so the device program is rank-agnostic.
"""

import numpy as np
from contextlib import ExitStack

N = 8192
D = 256
NCORES = 8
SHARD = N // NCORES          # 1024 rows per core
RB = 256                     # rows per row-block
NRB = SHARD // RB            # 4 row-blocks per core
NCH = N // 128               # 64 column chunks
NP = NCH // 2                # 32 chunk pairs
LN_EPS = 1e-5
SCALE = float(D) ** -0.5


def _build(m00, m01, m10, m11, debug=False):
    import concourse.bass as bass
    import concourse.tile as tile
    from concourse import bacc, mybir
    from concourse.masks import make_identity

    fp32 = mybir.dt.float32
    f32r = mybir.dt.float32r
    bf16 = mybir.dt.bfloat16
    AF = mybir.ActivationFunctionType
    ALU = mybir.AluOpType

    fast_mix = abs(m00 - m01) < 1e-30 and abs(m10 - m11) < 1e-30

    nc = bacc.Bacc("TRN2", target_bir_lowering=False, debug=False)

    h_d = nc.dram_tensor("h", [N, D], fp32, kind="ExternalInput").ap()
    x_d = nc.dram_tensor("x", [N, D], fp32, kind="ExternalInput").ap()
    wk_d = nc.dram_tensor("Wk", [D, D], fp32, kind="ExternalInput").ap()
    wq_d = nc.dram_tensor("Wq", [D, D], fp32, kind="ExternalInput").ap()
    wv_d = nc.dram_tensor("Wv", [D, D], fp32, kind="ExternalInput").ap()
    gam_d = nc.dram_tensor("ln_gamma", [D], fp32, kind="ExternalInput").ap()
    bet_d = nc.dram_tensor("ln_beta", [D], fp32, kind="ExternalInput").ap()
    hb_d = nc.dram_tensor("h_blk", [SHARD, D], fp32, kind="ExternalInput").ap()
    xb_d = nc.dram_tensor("x_blk", [SHARD, D], fp32, kind="ExternalInput").ap()
    hout_d = nc.dram_tensor("h_new", [SHARD, D], fp32, kind="ExternalOutput").ap()
    xout_d = nc.dram_tensor("x_new", [SHARD, D], fp32, kind="ExternalOutput").ap()

    # DRAM temps: transposed q and x, pair-major: [pair, d_sub, d_in, 2*128]
    qT_d = nc.dram_tensor("qT_tmp", [NP, 2, 128, 256], f32r, kind="Internal").ap()
    xT_d = nc.dram_tensor("xT_tmp", [NP, 2, 128, 256], f32r, kind="Internal").ap()

    dbg = {}
    if debug:
        for nm, shp in [("dbg_S", [128, RB]), ("dbg_G2", [128, RB]),
                        ("dbg_exp", [128, 512]), ("dbg_g", [128, 512]),
                        ("dbg_ah", [128, 512]), ("dbg_hn", [128, D]),
                        ("dbg_ph", [128, 2, RB]), ("dbg_h3", [128, 2, RB]),
                        ("dbg_bsh", [128, RB]), ("dbg_bgh", [128, RB])]:
            dbg[nm] = nc.dram_tensor(nm, shp, fp32, kind="ExternalOutput").ap()

    with tile.TileContext(nc) as tc, ExitStack() as ctx:
        # ---------------- resident pools ----------------
        res = ctx.enter_context(tc.tile_pool(name="res", bufs=1))
        hn_res = res.tile([128, NCH, D], bf16)     # HN natural, bf16
        xn_res = res.tile([128, NCH, D], bf16)     # x natural, bf16
        ktb = res.tile([128, 2, SHARD], f32r)      # kT of own rows
        xtb = res.tile([128, 2, SHARD], f32r)      # xT of own rows
        wq_sb = res.tile([128, 2, D], f32r)
        wk_sb = res.tile([128, 2, D], f32r)
        wv_sb = res.tile([128, 2, D], f32r)
        ones_bf = res.tile([128, 128], bf16)
        ident = res.tile([128, 128], fp32)
        gam_b = res.tile([128, D], fp32)
        bet_b = res.tile([128, D], fp32)

        eps_c = res.tile([128, 1], fp32)
        tiny_c = res.tile([128, 1], fp32)
        nc.vector.memset(eps_c[:], LN_EPS)
        nc.vector.memset(tiny_c[:], 1e-30)
        nc.vector.memset(ones_bf[:], 1.0)
        make_identity(nc, ident)
        # broadcast gamma/beta across partitions via partition-step-0 DMA
        gam_bc = bass.AP(tensor=gam_d.tensor, offset=gam_d.offset,
                         ap=[[0, 128]] + list(gam_d.ap))
        bet_bc = bass.AP(tensor=bet_d.tensor, offset=bet_d.offset,
                         ap=[[0, 128]] + list(bet_d.ap))
        nc.sync.dma_start(gam_b[:], gam_bc)
        nc.sync.dma_start(bet_b[:], bet_bc)

        # weights -> f32r
        with tc.tile_pool(name="wtmp", bufs=1) as wtmp:
            for wd, wsb in ((wq_d, wq_sb), (wk_d, wk_sb), (wv_d, wv_sb)):
                wt = wtmp.tile([128, 2, D], fp32, tag="wt")
                nc.sync.dma_start(wt[:], wd.rearrange("(s p) d -> p s d", p=128))
                nc.vector.tensor_copy(wsb[:], wt[:])

        # ---------------- prepass ----------------
        # LN stats for all 64 full-N chunks + 8 own-shard chunks in one
        # sweep, then a SINGLE batched Ln + Exp for every rstd (per-chunk
        # Ln/Exp thrashes the ACT table sets, ~2.7us per switch).
        NLN = NCH + SHARD // 128
        mv_res = res.tile([128, NLN, 2], fp32)
        rstd_res = res.tile([128, NLN], fp32)

        with nc.named_scope("pre_stats"), tc.tile_pool(name="lnstat", bufs=4) as lnp:
            for j in range(NLN):
                src = (h_d[j * 128:(j + 1) * 128, :] if j < NCH
                       else hb_d[(j - NCH) * 128:(j - NCH + 1) * 128, :])
                ht = lnp.tile([128, D], fp32, tag="ln_h")
                nc.sync.dma_start(ht[:], src)
                st = lnp.tile([128, nc.vector.BN_STATS_DIM], fp32, tag="ln_st")
                nc.vector.bn_stats(out=st[:], in_=ht[:])
                nc.vector.bn_aggr(out=mv_res[:, j, :], in_=st[:])
            lnt = lnp.tile([128, NLN], fp32, tag="ln_t")
            nc.scalar.activation(lnt[:], mv_res[:, :, 1], AF.Ln,
                                 bias=eps_c[:], scale=1.0)
            nc.scalar.activation(rstd_res[:], lnt[:], AF.Exp,
                                 bias=0.0, scale=-0.5)

        def ln_chunk(pool, src_ap, mvj, rstdj):
            """Re-load chunk and normalize with precomputed stats (GpSimd)."""
            ht = pool.tile([128, D], fp32, tag="ln_h")
            nc.sync.dma_start(ht[:], src_ap)
            cen = pool.tile([128, D], fp32, tag="ln_cen")
            nc.vector.scalar_tensor_tensor(
                cen[:], ht[:], mvj, gam_b[:], ALU.subtract, ALU.mult)
            hnf = pool.tile([128, D], fp32, tag="ln_out")
            nc.vector.scalar_tensor_tensor(
                hnf[:], cen[:], rstdj, bet_b[:], ALU.mult, ALU.add)
            return hnf

        with tc.tile_pool(name="pre", bufs=3) as pre, \
             tc.tile_pool(name="pre_win", bufs=2) as pre_win, \
             tc.tile_pool(name="pre_ps", bufs=2, space="PSUM") as pre_ps, \
             tc.tile_pool(name="pre_ps2", bufs=2, space="PSUM") as pre_ps2:

            def transpose_to(win_tile, i, fslice, src_fp32):
                """PE-transpose [128,128] fp32 -> f32r window slice."""
                pst = pre_ps.tile([128, 128], fp32, tag="tps")
                nc.tensor.transpose(pst[:], src_fp32, ident[:])
                nc.vector.tensor_copy(win_tile[:, i, fslice], pst[:])

            # full-N pass: HN resident + qT windows + xb resident + xT
            with nc.named_scope("pre_main"):
                for w in range(NCH // 4):
                    hnT_win = pre_win.tile([128, 2, 512], f32r, tag="hnT_win")
                    xT_win = pre_win.tile([128, 2, 512], f32r, tag="xT_win")
                    for jj in range(4):
                        j = w * 4 + jj
                        hnf = ln_chunk(pre, h_d[j * 128:(j + 1) * 128, :],
                                       mv_res[:, j, 0:1], rstd_res[:, j:j + 1])
                        nc.gpsimd.tensor_copy(hn_res[:, j, :], hnf[:])
                        fs = bass.ds(jj * 128, 128)
                        for i in range(2):
                            transpose_to(hnT_win, i, fs,
                                         hnf[:, i * 128:(i + 1) * 128])
                        xt = pre.tile([128, D], fp32, tag="x_in")
                        nc.sync.dma_start(xt[:], x_d[j * 128:(j + 1) * 128, :])
                        nc.gpsimd.tensor_copy(xn_res[:, j, :], xt[:])
                        for i in range(2):
                            transpose_to(xT_win, i, fs,
                                         xt[:, i * 128:(i + 1) * 128])
                    # qT = Wq^T @ hnT over this 512-col window
                    psq = pre_ps2.tile([128, 2, 512], fp32, tag="psq")
                    for p2 in range(2):
                        for i in range(2):
                            nc.tensor.matmul(
                                psq[:, p2, :], wq_sb[:, i, p2 * 128:(p2 + 1) * 128],
                                hnT_win[:, i, :], start=(i == 0), stop=(i == 1))
                    qt_sb = pre_win.tile([128, 2, 512], f32r, tag="qt_sb")
                    nc.vector.tensor_copy(qt_sb[:], psq[:])
                    for jj in range(2):
                        jp = w * 2 + jj
                        sl = bass.ds(jj * 256, 256)
                        nc.sync.dma_start(
                            qT_d[jp].rearrange("s p n -> p s n"), qt_sb[:, :, sl])
                        nc.sync.dma_start(
                            xT_d[jp].rearrange("s p n -> p s n"), xT_win[:, :, sl])

            # own-shard pass: kTb (needs LN of h_blk) and xTb
            with nc.named_scope("pre_blk"):
                for w in range(SHARD // 512):
                    hnT_win = pre_win.tile([128, 2, 512], f32r, tag="hnT_win")
                    for jj in range(4):
                        j = w * 4 + jj
                        hnf = ln_chunk(pre, hb_d[j * 128:(j + 1) * 128, :],
                                       mv_res[:, NCH + j, 0:1],
                                       rstd_res[:, NCH + j:NCH + j + 1])
                        fs = bass.ds(jj * 128, 128)
                        for i in range(2):
                            transpose_to(hnT_win, i, fs,
                                         hnf[:, i * 128:(i + 1) * 128])
                        xt = pre.tile([128, D], fp32, tag="x_in")
                        nc.sync.dma_start(xt[:], xb_d[j * 128:(j + 1) * 128, :])
                        for i in range(2):
                            transpose_to(xtb, i, bass.ds(j * 128, 128),
                                         xt[:, i * 128:(i + 1) * 128])
                    psq = pre_ps2.tile([128, 2, 512], fp32, tag="psq")
                    for p2 in range(2):
                        for i in range(2):
                            nc.tensor.matmul(
                                psq[:, p2, :], wk_sb[:, i, p2 * 128:(p2 + 1) * 128],
                                hnT_win[:, i, :], start=(i == 0), stop=(i == 1))
                    nc.vector.tensor_copy(ktb[:, :, w * 512:(w + 1) * 512], psq[:])

        # ---------------- main loop ----------------
        exp_pool = ctx.enter_context(tc.tile_pool(name="exp_strip", bufs=NP))
        g_pool = ctx.enter_context(tc.tile_pool(name="g_strip", bufs=NP))
        stream = ctx.enter_context(tc.tile_pool(name="stream", bufs=4))
        resid = ctx.enter_context(tc.tile_pool(name="resid", bufs=2))
        work = ctx.enter_context(tc.tile_pool(name="work", bufs=4))
        mixp = ctx.enter_context(tc.tile_pool(name="mixp", bufs=2))
        tailp = ctx.enter_context(tc.tile_pool(name="tailp", bufs=1))
        bpool = ctx.enter_context(tc.tile_pool(name="btiles", bufs=2))
        outp = ctx.enter_context(tc.tile_pool(name="outp", bufs=2))
        ps_sem = ctx.enter_context(tc.tile_pool(name="ps_sem", bufs=2, space="PSUM"))
        ps_gr = ctx.enter_context(tc.tile_pool(name="ps_gr", bufs=2, space="PSUM"))
        ps_st = ctx.enter_context(tc.tile_pool(name="ps_st", bufs=1, space="PSUM"))
        ps_ap = ctx.enter_context(tc.tile_pool(name="ps_ap", bufs=1, space="PSUM"))
        ps_tl = ctx.enter_context(tc.tile_pool(name="ps_tl", bufs=1, space="PSUM"))

        if debug:
            dbgp = ctx.enter_context(tc.tile_pool(name="dbgp", bufs=2))

            def dump(name, src_ap, shape):
                t = dbgp.tile(shape, fp32, tag="dump")
                nc.vector.tensor_copy(t[:], src_ap)
                nc.sync.dma_start(dbg[name], t[:])

            dump("dbg_hn", hn_res[:, 0, :], [128, D])

        for rb in range(NRB):
            rsl = bass.ds(rb * RB, RB)        # row slice within shard

            # ---- pass 1: scores, exp, square, stats (chunk pairs) ----
            e_tiles, gt_tiles = [], []
            ps_stat = ps_st.tile([128, 2, RB], fp32, tag="stat")
            with nc.named_scope(f"rb{rb}_p1"):
                for jp in range(NP):
                    qt = stream.tile([128, 2, 256], f32r, tag="qt")
                    nc.sync.dma_start(qt[:], qT_d[jp].rearrange("s p n -> p s n"))
                    xt = stream.tile([128, 2, 256], f32r, tag="xt")
                    nc.sync.dma_start(xt[:], xT_d[jp].rearrange("s p n -> p s n"))

                    psem = ps_sem.tile([128, 2, RB], fp32, tag="sem")
                    pgr = ps_gr.tile([128, 2, RB], fp32, tag="gr")
                    for jj in range(2):
                        for i in range(2):
                            nc.tensor.matmul(
                                psem[:, jj, :], qt[:, i, jj * 128:(jj + 1) * 128],
                                ktb[:, i, rsl], start=(jj == 0 and i == 0),
                                stop=(jj == 1 and i == 1), skip_group_check=True)
                    for jj in range(2):
                        for i in range(2):
                            nc.tensor.matmul(
                                pgr[:, jj, :], xt[:, i, jj * 128:(jj + 1) * 128],
                                xtb[:, i, rsl], start=(jj == 0 and i == 0),
                                stop=(jj == 1 and i == 1), skip_group_check=True)

                    et = exp_pool.tile([128, 512], bf16, tag="exp")
                    nc.scalar.activation(et[:], psem[:, :, :], AF.Exp,
                                         bias=0.0, scale=SCALE)
                    gt = g_pool.tile([128, 512], bf16, tag="g")
                    nc.vector.tensor_copy(gt[:], pgr[:, :, :])
                    sq = work.tile([128, 512], bf16, tag="sq")
                    nc.scalar.activation(sq[:], pgr[:, :, :], AF.Square)

                    # stats: S / G^2 broadcast over partitions. Both halves
                    # of ps_stat share one bank: only the very first matmul
                    # may set start=True (start clears the whole bank).
                    for jj in range(2):
                        nc.tensor.matmul(
                            ps_stat[:, 0, :], ones_bf[:],
                            et[:, jj * 256:(jj + 1) * 256],
                            start=(jp == 0 and jj == 0),
                            stop=(jp == NP - 1 and jj == 1),
                            skip_group_check=True)
                        nc.tensor.matmul(
                            ps_stat[:, 1, :], ones_bf[:],
                            sq[:, jj * 256:(jj + 1) * 256],
                            start=False,
                            stop=(jp == NP - 1 and jj == 1),
                            skip_group_check=True)
                    if debug and rb == 0 and jp == 0:
                        dump("dbg_exp", et[:], [128, 512])
                        dump("dbg_g", gt[:], [128, 512])
                    e_tiles.append(et)
                    gt_tiles.append(gt)

            # ---- mix prep: B tiles ----
            with nc.named_scope(f"rb{rb}_mix"):
                b_sh = bpool.tile([128, RB], fp32, tag="b_sh")
                b_gh = bpool.tile([128, RB], fp32, tag="b_gh")
                rec = mixp.tile([128, RB], fp32, tag="rec")
                nc.vector.reciprocal(rec[:], ps_stat[:, 0, :])
                nc.vector.tensor_scalar_mul(b_sh[:], rec[:], float(m00))
                lng = mixp.tile([128, RB], fp32, tag="lng")
                nc.scalar.activation(lng[:], ps_stat[:, 1, :], AF.Ln,
                                     bias=tiny_c[:], scale=1.0)
                ginv = mixp.tile([128, RB], fp32, tag="ginv")
                nc.scalar.activation(ginv[:], lng[:], AF.Exp, bias=0.0, scale=-0.5)
                nc.vector.tensor_scalar_mul(b_gh[:], ginv[:], float(m10))
                if not fast_mix:
                    b_sx = bpool.tile([128, RB], fp32, tag="b_sx")
                    b_gx = bpool.tile([128, RB], fp32, tag="b_gx")
                    nc.vector.tensor_scalar_mul(b_sx[:], rec[:], float(m01))
                    nc.vector.tensor_scalar_mul(b_gx[:], ginv[:], float(m11))
                if debug and rb == 0:
                    dump("dbg_S", ps_stat[:, 0, :], [128, RB])
                    dump("dbg_G2", ps_stat[:, 1, :], [128, RB])
                    dump("dbg_bsh", b_sh[:], [128, RB])
                    dump("dbg_bgh", b_gh[:], [128, RB])

            # ---- pass 2: mix + apply (chunk pairs) ----
            ph = ps_ap.tile([128, 2, RB], fp32, tag="ph")
            px = ps_ap.tile([128, 2, RB], fp32, tag="px")
            b_sh2 = b_sh[:, None, :].to_broadcast([128, 2, RB])
            b_gh2 = b_gh[:, None, :].to_broadcast([128, 2, RB])
            if not fast_mix:
                b_sx2 = b_sx[:, None, :].to_broadcast([128, 2, RB])
                b_gx2 = b_gx[:, None, :].to_broadcast([128, 2, RB])
            with nc.named_scope(f"rb{rb}_p2"):
                for jp in range(NP):
                    et, gt = e_tiles[jp], gt_tiles[jp]
                    et2 = et[:].rearrange("p (a b) -> p a b", a=2)
                    gt2 = gt[:].rearrange("p (a b) -> p a b", a=2)
                    t1 = work.tile([128, 2, RB], bf16, tag="t1")
                    nc.vector.tensor_tensor(t1[:], et2, b_sh2, ALU.mult)
                    t2 = work.tile([128, 2, RB], bf16, tag="t2")
                    nc.gpsimd.tensor_tensor(t2[:], gt2, b_gh2, ALU.mult)
                    ah = et  # reuse exp slot for mixed A_h
                    if not fast_mix:
                        t1x = work.tile([128, 2, RB], bf16, tag="t1x")
                        nc.vector.tensor_tensor(t1x[:], et2, b_sx2, ALU.mult)
                        t2x = work.tile([128, 2, RB], bf16, tag="t2x")
                        nc.gpsimd.tensor_tensor(t2x[:], gt2, b_gx2, ALU.mult)
                        ax = gt
                        nc.vector.tensor_tensor(
                            ah[:].rearrange("p (a b) -> p a b", a=2),
                            t1[:], t2[:], ALU.add)
                        nc.vector.tensor_tensor(
                            ax[:].rearrange("p (a b) -> p a b", a=2),
                            t1x[:], t2x[:], ALU.add)
                    else:
                        ax = ah
                        nc.vector.tensor_tensor(
                            ah[:].rearrange("p (a b) -> p a b", a=2),
                            t1[:], t2[:], ALU.add)
                    if debug and rb == 0 and jp == 0:
                        dump("dbg_ah", ah[:], [128, 512])

                    # ph/px halves share a bank: first-touch start only.
                    for jj in range(2):
                        j = jp * 2 + jj
                        asl = bass.ds(jj * 256, 256)
                        for i in range(2):
                            nc.tensor.matmul(
                                ph[:, i, :], hn_res[:, j, i * 128:(i + 1) * 128],
                                ah[:, asl],
                                start=(jp == 0 and jj == 0 and i == 0),
                                stop=(jp == NP - 1 and jj == 1),
                                skip_group_check=True)
                        for i in range(2):
                            nc.tensor.matmul(
                                px[:, i, :], xn_res[:, j, i * 128:(i + 1) * 128],
                                ax[:, asl],
                                start=(jp == 0 and jj == 0 and i == 0),
                                stop=(jp == NP - 1 and jj == 1),
                                skip_group_check=True)

            # ---- tail: h path (@Wv, ELU, residual) ----
            with nc.named_scope(f"rb{rb}_tail"):
                if debug and rb == 0:
                    dump("dbg_ph", ph[:], [128, 2, RB])
                o2t = tailp.tile([128, 2, RB], f32r, tag="o2t")
                nc.vector.tensor_copy(o2t[:], ph[:])
                ps_h3 = ps_tl.tile([128, 2, RB], fp32, tag="tl")
                for rc in range(2):
                    for dc in range(2):
                        nc.tensor.matmul(
                            ps_h3[:, rc, :], o2t[:, dc, rc * 128:(rc + 1) * 128],
                            wv_sb[:, dc, :], start=(rc == 0 and dc == 0),
                            stop=(rc == 1 and dc == 1), skip_group_check=True)
                if debug and rb == 0:
                    dump("dbg_h3", ps_h3[:], [128, 2, RB])
                for rc in range(2):
                    rows = bass.ds(rb * RB + rc * 128, 128)
                    h0 = resid.tile([128, D], fp32, tag="h0")
                    nc.sync.dma_start(h0[:], hb_d[rows, :])
                    tmin = tailp.tile([128, D], fp32, tag="tmin")
                    nc.vector.tensor_scalar_min(tmin[:], ps_h3[:, rc, :], 0.0)
                    ee = tailp.tile([128, D], fp32, tag="ee")
                    nc.scalar.activation(ee[:], tmin[:], AF.Exp)
                    uu = tailp.tile([128, D], fp32, tag="uu")
                    nc.vector.tensor_scalar_max(uu[:], ps_h3[:, rc, :], 0.0)
                    s1 = tailp.tile([128, D], fp32, tag="s1")
                    nc.vector.scalar_tensor_tensor(
                        s1[:], ee[:], -1.0, uu[:], ALU.add, ALU.add)
                    ho = outp.tile([128, D], fp32, tag="hout")
                    nc.vector.tensor_tensor(ho[:], s1[:], h0[:], ALU.add)
                    nc.sync.dma_start(hout_d[rows, :], ho[:])

                # ---- tail: x path (transpose back, residual) ----
                oxt = tailp.tile([128, 2, RB], fp32, tag="oxt")
                nc.vector.tensor_copy(oxt[:], px[:])
                for rc in range(2):
                    rows = bass.ds(rb * RB + rc * 128, 128)
                    x0 = resid.tile([128, D], fp32, tag="x0")
                    nc.sync.dma_start(x0[:], xb_d[rows, :])
                    xo = outp.tile([128, D], fp32, tag="xout")
                    for dc in range(2):
                        ps_tr = ps_tl.tile([128, 128], fp32, tag="tl")
                        nc.tensor.transpose(
                            ps_tr[:], oxt[:, dc, rc * 128:(rc + 1) * 128], ident[:])
                        nc.vector.tensor_tensor(
                            xo[:, dc * 128:(dc + 1) * 128], ps_tr[:],
                            x0[:, dc * 128:(dc + 1) * 128], ALU.add)
                    nc.sync.dma_start(xout_d[rows, :], xo[:])

    nc.compile()
    return nc


_CACHE = {}


def kernel(h, x, Wk, Wq, Wv, ln_gamma, ln_beta, mixing):
    from concourse import bass_utils

    h = np.ascontiguousarray(h, dtype=np.float32)
    x = np.ascontiguousarray(x, dtype=np.float32)
    Wk = np.ascontiguousarray(Wk, dtype=np.float32)
    Wq = np.ascontiguousarray(Wq, dtype=np.float32)
    Wv = np.ascontiguousarray(Wv, dtype=np.float32)
    ln_gamma = np.ascontiguousarray(ln_gamma, dtype=np.float32)
    ln_beta = np.ascontiguousarray(ln_beta, dtype=np.float32)
    mixing = np.asarray(mixing, dtype=np.float64)

    # softmax over dim 0 of the 2x2 mixing (4 scalars -> compile-time consts)
    mx = mixing - mixing.max(axis=0, keepdims=True)
    em = np.exp(mx)
    m = em / em.sum(axis=0, keepdims=True)
    m00, m01, m10, m11 = float(m[0, 0]), float(m[0, 1]), float(m[1, 0]), float(m[1, 1])

    key = (m00, m01, m10, m11)
    if key not in _CACHE:
        _CACHE[key] = _build(m00, m01, m10, m11)
    nc = _CACHE[key]

    in_maps = []
    for c in range(NCORES):
        sl = slice(c * SHARD, (c + 1) * SHARD)
        in_maps.append({
            "h": h, "x": x, "Wk": Wk, "Wq": Wq, "Wv": Wv,
            "ln_gamma": ln_gamma, "ln_beta": ln_beta,
            "h_blk": np.ascontiguousarray(h[sl]),
            "x_blk": np.ascontiguousarray(x[sl]),
        })

    res = bass_utils.run_bass_kernel_spmd(
        nc, in_maps, core_ids=list(range(NCORES)))

    h_new = np.concatenate([res.results[c]["h_new"] for c in range(NCORES)], axis=0)
    x_new = np.concatenate([res.results[c]["x_new"] for c in range(NCORES)], axis=0)
    return (h_new, x_new)
